# revision 70
# baseline (speedup 1.0000x reference)
"""Multi-head self-attention on 8 trn2 NeuronCores — fp8 DoubleRow edition.

Problem: x[2,2048,1024], 16 heads, depth 64; out = MHA(x) with QKV/O
projections (reference.py / nn_MultiHeadSelfAttention_3341484556968).

Sharding: tensor-parallel over heads; core c owns heads {2c, 2c+1}.

Key ideas vs the fp32r baseline (286us):
  - All heavy matmuls use fp8e4m3 operands in DoubleRow perf mode
    (0.5 cycles/output-column, i.e. 4x the fp32r PE rate):
      * QKV projections: contraction over D=1024 as 4 DoubleRow steps
        of K=256, three passes each (x8*W8 + rx8*W8 + x8*rW8) with e5m2
        residual tensors, so only the fp8 re-quantization of Q/K before
        the scores matmul costs accuracy. Each 256-column psum half is
        one sequential accumulation chain (one pending group per bank).
      * Scores: K-side stationary with a stride-0 "slot" dim duplicating
        the K=64 contraction (result is 2x scores, folded into the exp
        scale). No partition repacking needed.
  - exp on ScalarE writes bf16 (ACT cost is dtype-blind); its ~133us
    stream is the bottleneck the whole schedule is shaped around: PV
    trails the score/exp stream by two key-chunk pairs and all other PE
    work is sliced small between score groups (the PE is in-order).
  - PV runs in bf16 in the transposed baseline orientation (stationary
    V16 [keys, 65] with an appended ones column, one psum accumulation
    group per head bank; softmax denominators land in row 64).
  - Weights are host-scaled by 16 (fp8-friendly range); the scale folds
    into the exp scale (Q,K) and into Wo (V). Wk/Wv biases fold away
    mathematically (softmax shift invariance; bo' = bo + Wo@bv).
  - Normalization on the sender (reciprocal + rank-1 broadcast matmul),
    bf16 AllToAll payloads, deferred stages, batch-outer bf16 output
    projection so batch 0's half overlaps batch 1's collective.
"""

import os
import numpy as np
import ml_dtypes

import concourse.bacc as bacc
import concourse.mybir as mybir
import concourse.tile as tile

F32 = mybir.dt.float32
F32R = mybir.dt.float32r
BF16 = mybir.dt.bfloat16
F8E4 = mybir.dt.float8e4
F8E5 = mybir.dt.float8e5
U8 = mybir.dt.uint8
AF = mybir.ActivationFunctionType
DR = mybir.MatmulPerfMode.DoubleRow

P = 128          # partitions / PE contraction width
DUPF = 2.0       # stride-0 slot duplication factor in the scores DR matmul
WSCALE = 16.0    # host scaling of Wq/Wk/Wv for fp8 range

E4 = ml_dtypes.float8_e4m3
E5 = ml_dtypes.float8_e5m2
BF = ml_dtypes.bfloat16


def build_nc(B=2, S=2048, D=1024, H=16, ncores=8):
    DEP = D // H                 # head depth (64)
    HPC = H // ncores            # heads per core (2)
    FPC = HPC * DEP              # features per core (128)
    R = B * S                    # flattened rows (4096)
    RC = R // ncores             # output rows per core (512)
    KD = D // P                  # contraction chunks for projections (8)
    KJ = KD // 2                 # DoubleRow K=256 steps (4)
    RWC = 512                    # row chunk for projections (per batch)
    NRWB = S // RWC              # projection row-chunks per batch (4)
    QCH = 512                    # query columns per block
    NQC = S // QCH               # q blocks per batch (4)
    NKC = S // P                 # key chunks per batch (16)
    NJP = NKC // 2               # key chunk pairs per batch (8)
    NDO = D // P                 # output-feature chunks (8)
    SC = S // ncores             # per-batch rows per core (256)
    assert FPC == P
    # psum scores carry 2*WSCALE^2; exp applies the real 1/sqrt(DEP)
    scale_exp = 1.0 / (np.sqrt(DEP) * DUPF * WSCALE * WSCALE)

    nc = bacc.Bacc("TRN2", target_bir_lowering=False, debug=False,
                   num_devices=ncores)

    x8T = nc.dram_tensor("x8T", [D, R], F8E4, kind="ExternalInput")
    rx8T = nc.dram_tensor("rx8T", [D, R], F8E5, kind="ExternalInput")
    w8 = nc.dram_tensor("w8", [D, 3 * FPC], F8E4, kind="ExternalInput")
    rw8 = nc.dram_tensor("rw8", [D, 3 * FPC], F8E5, kind="ExternalInput")
    bq16 = nc.dram_tensor("bq16", [FPC, 1], F32, kind="ExternalInput")
    woT = nc.dram_tensor("woT", [D, D], BF16, kind="ExternalInput")
    bo2 = nc.dram_tensor("bo2", [P, NDO], F32, kind="ExternalInput")
    identb = nc.dram_tensor("identb", [P, P], BF16, kind="ExternalInput")
    outT = nc.dram_tensor("outT", [D, RC], F32, kind="ExternalOutput")

    with tile.TileContext(nc) as tc:
        with (
            tc.tile_pool(name="persist", bufs=1) as persist,
            tc.tile_pool(name="stream", bufs=2) as stream,
            tc.tile_pool(name="work", bufs=2) as work,
            tc.tile_pool(name="dram", bufs=1, space="DRAM") as dram,
        ):
            # ---- constants / weights resident in SBUF ----
            # critical path first on the SP queue (x chunk 0, then Q/K
            # weights); everything else rides the gpsimd SWDGE queue
            w8_sb = persist.tile([P, KD, 3 * FPC], F8E4)
            bq_sb = persist.tile([FPC, 1], F32)
            rw8_sb = persist.tile([P, KD, 3 * FPC], F8E5)
            bo_sb = persist.tile([P, NDO], F32)
            ident_sb = persist.tile([P, P], BF16)

            def emit_const_loads():
                # Q/K weights ride the ACT hwdge queue in parallel with the
                # SP queue's first x chunk — both gate the first exp
                w8r = w8.ap().rearrange("(ko p) m -> p ko m", p=P)
                nc.scalar.dma_start(w8_sb[:, :, 0:2 * FPC],
                                    w8r[:, :, 0:2 * FPC])
                nc.gpsimd.dma_start(
                    rw8_sb, rw8.ap().rearrange("(ko p) m -> p ko m", p=P))
                nc.scalar.dma_start(bq_sb, bq16.ap())
            wo_all = persist.tile([P, NDO, D], BF16)

            def emit_wo_loads():
                # deferred: needed only at output projection time
                for i in range(NDO):
                    nc.gpsimd.dma_start(wo_all[:, i, :],
                                        woT.ap()[i * P:(i + 1) * P, :])

            QT8 = persist.tile([P, R], F8E4)
            KT8 = persist.tile([P, R], F8E4)
            # V16: [keys, chunk, head, DEP+1] bf16; col DEP is the ones
            # column feeding softmax denominators
            NCT = B * NKC        # total key chunks (32)
            V16 = persist.tile([P, NCT, HPC, DEP + 1], BF16)
            nc.vector.memset(
                V16[:, :, :, DEP:DEP + 1].bitcast(mybir.dt.uint16), 0x3F80)
            ones_col = persist.tile([1, DEP], F32R)
            nc.vector.memset(ones_col.bitcast(mybir.dt.uint32), 0x3F800000)

            chunk_sb = [persist.tile([P, NDO, SC], BF16, name=f"chunk_{b}")
                        for b in range(B)]
            a2a_in = [dram.tile([ncores, FPC, SC], BF16,
                                name=f"a2a_in_{b}") for b in range(B)]
            a2a_out = [dram.tile([ncores, FPC, SC], BF16,
                                 name=f"a2a_out_{b}") for b in range(B)]

            psd = tc.tile_pool(name="ps_bcd", bufs=1, space="PSUM")
            ps = psd.__enter__()

            xs_tiles = {}

            def emit_xload(b, rwb):
                # x8 on the SP DGE queue; the e5m2 residual rides the gpsimd
                # queue so the two streams transfer in parallel
                r0 = b * S + rwb * RWC
                x8s = stream.tile([P, KD, RWC], F8E4, tag="x8s", bufs=3,
                                  name=f"x8s_{b}_{rwb}")
                rx8s = stream.tile([P, KD, RWC], F8E5, tag="rx8s", bufs=3,
                                   name=f"rx8s_{b}_{rwb}")
                src = x8T.ap()[:, r0:r0 + RWC].rearrange(
                    "(ko p) n -> p ko n", p=P)
                rsrc = rx8T.ap()[:, r0:r0 + RWC].rearrange(
                    "(ko p) n -> p ko n", p=P)
                nc.sync.dma_start(x8s, src)
                nc.gpsimd.dma_start(rx8s, rsrc)
                xs_tiles[(b, rwb)] = (x8s, rx8s)

            def dr_proj(pq, lhs_w, rhs_x, j, first, last):
                # one DoubleRow K=256 step over both 256-col halves
                for half in range(2):
                    nc.tensor.matmul(
                        pq[:, half * 256:(half + 1) * 256],
                        lhs_w, rhs_x[:, 2 * j:2 * j + 2,
                                     half * 256:(half + 1) * 256],
                        start=first, stop=last and half == 1,
                        perf_mode=DR)

            def emit_proj_one(b, rwb, t, c0=0, c1=RWC):
                # t: 0=Q, 1=K; three DoubleRow passes (x8*W8 + rx8*W8 +
                # x8*rW8) so only the fp8 re-quantization before the
                # scores matmul costs accuracy; [c0:c1) selects a row range
                r0 = b * S + rwb * RWC
                x8s, rx8s = xs_tiles[(b, rwb)]
                dst = (QT8, KT8)[t]
                pq = ps.tile([P, RWC], F32, tag="proj", bufs=2,
                             name=f"pqk_{b}_{rwb}_{t}_{c0}")
                cs = slice(t * FPC, (t + 1) * FPC)
                # one accumulation chain per 256-column half (a single
                # psum bank only supports one pending group at a time)
                for h0 in range(c0, c1, 256):
                    hw_ = min(256, c1 - h0)
                    for pi, (wsb, xsb) in enumerate(
                            [(w8_sb, x8s), (w8_sb, rx8s), (rw8_sb, x8s)]):
                        for j in range(KJ):
                            nc.tensor.matmul(
                                pq[:, h0:h0 + hw_],
                                wsb[:, 2 * j:2 * j + 2, cs],
                                xsb[:, 2 * j:2 * j + 2, h0:h0 + hw_],
                                start=(pi == 0 and j == 0),
                                stop=(pi == 2 and j == KJ - 1),
                                perf_mode=DR)
                if t == 0:
                    nc.vector.tensor_scalar_add(
                        dst[:, r0 + c0:r0 + c1], pq[:, c0:c1], bq_sb)
                else:
                    nc.vector.tensor_copy(dst[:, r0 + c0:r0 + c1],
                                          pq[:, c0:c1])

            def emit_proj_qk(b, rwb):
                emit_proj_one(b, rwb, 0)
                emit_proj_one(b, rwb, 1)

            pv_psum = {}

            def emit_proj_v_half(b, rwb, half):
                x8s, rx8s = xs_tiles[(b, rwb)]
                if half == 0:
                    pv_psum[(b, rwb)] = ps.tile([P, RWC], F32, tag="proj",
                                                bufs=2, name=f"pv_{b}_{rwb}")
                pv = pv_psum[(b, rwb)]
                for pi, (wsb, xsb) in enumerate(
                        [(w8_sb, x8s), (w8_sb, rx8s), (rw8_sb, x8s)]):
                    for j in range(KJ):
                        nc.tensor.matmul(
                            pv[:, half * 256:(half + 1) * 256],
                            wsb[:, 2 * j:2 * j + 2, 2 * FPC:3 * FPC],
                            xsb[:, 2 * j:2 * j + 2,
                                half * 256:(half + 1) * 256],
                            start=(pi == 0 and j == 0),
                            stop=(pi == 2 and j == KJ - 1),
                            perf_mode=DR)
                if half == 1:
                    pv_psum.pop((b, rwb))
                    vt = work.tile([P, RWC], BF16, tag="vt16", bufs=3,
                                   name=f"vt16_{b}_{rwb}")
                    nc.vector.tensor_copy(vt, pv)
                    vt_tiles[(b, rwb)] = vt

            def emit_proj_v(b, rwb):
                for half in range(2):
                    emit_proj_v_half(b, rwb, half)

            vt_tiles = {}

            def emit_vtrans(b, rwb):
                # two key-chunk pairs per row chunk; each pair: two PE
                # transposes into one psum tile, then one quantize copy to
                # V8 and one subtract into rV8 (both heads in one op).
                # tp tiles share the "proj" psum slot (padded to 2KB).
                vt = vt_tiles.pop((b, rwb))
                for jj in range(2):
                    t0 = b * NKC + rwb * 4 + 2 * jj
                    tpw = ps.tile([P, 2, 4 * P], BF16, tag="proj", bufs=2,
                                  name=f"vtr_{b}_{rwb}_{jj}")
                    tp = tpw[:, :, 0:P]
                    for s in range(2):
                        nc.tensor.transpose(
                            tp[:, s, :],
                            vt[:, (2 * jj + s) * P:(2 * jj + s + 1) * P],
                            ident_sb)
                    src = tp.rearrange("p s (h d) -> p s h d", h=HPC)
                    nc.vector.tensor_copy(V16[:, t0:t0 + 2, :, 0:DEP], src)

            sc_tiles = {}
            ex_tiles = {}

            def emit_scores(b, qc, kc):
                g0 = b * S + qc * QCH
                k0 = b * S + kc * P
                sc = ps.tile([P, HPC, QCH], F32, tag="sc", bufs=2,
                             name=f"sc_{b}_{qc}_{kc}")
                sc_tiles[(b, qc, kc)] = sc
                for h in range(HPC):
                    lhs = KT8[h * DEP:(h + 1) * DEP, k0:k0 + P] \
                        .unsqueeze(1).broadcast_to([DEP, 2, P])
                    for half in range(2):
                        rhs = QT8[h * DEP:(h + 1) * DEP,
                                  g0 + half * 256:g0 + (half + 1) * 256] \
                            .unsqueeze(1).broadcast_to([DEP, 2, 256])
                        nc.tensor.matmul(
                            sc[:, h, half * 256:(half + 1) * 256],
                            lhs, rhs, start=True, stop=True, perf_mode=DR)

            def emit_exp(b, qc, kc):
                sc = sc_tiles.pop((b, qc, kc))
                jj = kc // 2
                if kc % 2 == 0:
                    ex_tiles[(b, qc, jj)] = work.tile(
                        [P, 2, HPC, QCH], BF16, tag="ex", bufs=7,
                        name=f"ex_{b}_{qc}_{jj}")
                ex = ex_tiles[(b, qc, jj)]
                nc.scalar.activation(ex[:, kc % 2, :, :], sc, AF.Exp,
                                     scale=float(scale_exp))

            attn_ps = {}

            def emit_pv(b, qc, jj):
                # attn accumulates transposed ([feat|denom, query]) with a
                # single psum group per head bank, like the fp32r baseline
                if jj == 0:
                    attn_ps[(b, qc)] = [
                        ps.tile([DEP + 1, QCH], F32, tag=f"attn{h}",
                                bufs=1, name=f"attn_{b}_{qc}_{h}")
                        for h in range(HPC)]
                ex = ex_tiles.pop((b, qc, jj))
                ap = attn_ps[(b, qc)]
                for s2 in range(2):
                    kc = 2 * jj + s2
                    t = b * NKC + kc
                    for h in range(HPC):
                        nc.tensor.matmul(
                            ap[h], V16[:, t, h, :], ex[:, s2, h, :],
                            start=(kc == 0), stop=(kc == NKC - 1))

            def emit_stage(b, qc):
                # normalize on the sender: reciprocal of the denominator
                # row, rank-1 broadcast via PE, multiply, stage shards
                ap = attn_ps.pop((b, qc))
                asb = work.tile([P, HPC, QCH], BF16, tag="asb", bufs=4,
                                name=f"asb_{b}_{qc}")
                for h in range(HPC):
                    rec = work.tile([1, QCH], F32R, tag="rec", bufs=4,
                                    name=f"rec_{b}_{qc}_{h}")
                    with nc.allow_low_precision(reason="softmax recip"):
                        nc.vector.reciprocal(rec, ap[h][DEP:DEP + 1, :])
                    bc = ps.tile([DEP, QCH], F32, tag="proj", bufs=2,
                                 name=f"bc_{b}_{qc}_{h}")
                    nc.tensor.matmul(bc, ones_col, rec, start=True,
                                     stop=True)
                    bcs = work.tile([DEP, QCH], F32, tag="bcs", bufs=2,
                                    name=f"bcs_{b}_{qc}_{h}")
                    nc.vector.tensor_copy(bcs, bc)
                    nc.vector.tensor_tensor(
                        asb[0:DEP, h, :], ap[h][0:DEP, :], bcs,
                        mybir.AluOpType.mult)
                ai = a2a_in[b]
                for half in range(2):
                    j = 2 * qc + half
                    cs = slice(half * SC, (half + 1) * SC)
                    nc.sync.dma_start(
                        ai[j, :, :].rearrange("(h d) n -> d h n", h=HPC),
                        asb[0:DEP, :, cs])

            def emit_collective(b):
                nc.gpsimd.collective_compute(
                    "AllToAll", mybir.AluOpType.bypass,
                    replica_groups=[list(range(ncores))],
                    ins=[a2a_in[b].opt()], outs=[a2a_out[b].opt()])

            def emit_chunk_load(b):
                # one DMA: all output-projection matmuls then wait on a
                # single semaphore value and stream back-to-back (a split
                # load gives each group a different wait and the tail
                # matmuls dispatch too slowly to keep the PE ramped)
                src = a2a_out[b].rearrange("i p n -> p i n")
                if b < B - 1:
                    nc.gpsimd.dma_start(chunk_sb[b], src)
                else:
                    nc.sync.dma_start(chunk_sb[b], src)

            # ---------------- schedule ----------------
            # The exp stream on ScalarE is the bottleneck. The PE is
            # in-order, so every instruction emitted between two score
            # groups delays the exp stream by its PE time; all non-score
            # PE work is sliced small and balanced across the kc slots.
            # qc0's PV pairs and every stage ride one qc behind.
            emit_const_loads()
            emit_xload(0, 0)
            emit_proj_one(0, 0, 1)                # K chunk 0
            emit_proj_one(0, 0, 0)                # Q chunk 0 (qc0 scores)
            nc.scalar.dma_start(
                w8_sb[:, :, 2 * FPC:3 * FPC],
                w8.ap().rearrange("(ko p) m -> p ko m", p=P)[
                    :, :, 2 * FPC:3 * FPC])
            nc.gpsimd.dma_start(bo_sb, bo2.ap())
            nc.gpsimd.dma_start(ident_sb, identb.ap())
            # chunk 0: all four score/exp groups contiguous so the first
            # exps' coarse engine-tick waits cover nothing but scores
            emit_xload(0, 1)
            for kk in range(4):
                emit_scores(0, 0, kk)
                emit_exp(0, 0, kk)
            emit_proj_one(0, 1, 0)                # Q chunk 1
            emit_proj_one(0, 1, 1)                # K chunk 1
            for rwb in range(1, NRWB):
                if rwb + 1 < NRWB:
                    emit_xload(0, rwb + 1)
                emit_scores(0, 0, 4 * rwb + 0)
                emit_exp(0, 0, 4 * rwb + 0)
                emit_proj_v_half(0, rwb - 1, 0)
                emit_scores(0, 0, 4 * rwb + 1)
                emit_exp(0, 0, 4 * rwb + 1)
                emit_proj_v_half(0, rwb - 1, 1)
                emit_scores(0, 0, 4 * rwb + 2)
                emit_exp(0, 0, 4 * rwb + 2)
                emit_vtrans(0, rwb - 1)
                emit_scores(0, 0, 4 * rwb + 3)
                emit_exp(0, 0, 4 * rwb + 3)
                if rwb + 1 < NRWB:
                    emit_proj_one(0, rwb + 1, 0)  # Q prefetch
                    emit_proj_one(0, rwb + 1, 1)  # K prefetch


            # b1 projections as small filler slices for qc2-3
            fill_q = []
            for rwb in range(NRWB):
                if B > 1:
                    fill_q.append(lambda r=rwb: (
                        emit_xload(1, r), emit_proj_one(1, r, 0)))
                    fill_q.append(lambda r=rwb: emit_proj_one(1, r, 1))
                    fill_q.append(lambda r=rwb: emit_proj_v_half(1, r, 0))
                    fill_q.append(lambda r=rwb: emit_proj_v_half(1, r, 1))
                    fill_q.append(lambda r=rwb: emit_vtrans(1, r))

            def emit_filler_slice():
                if fill_q:
                    fill_q.pop(0)()

            stage_prev = []

            def emit_stage_prev():
                if stage_prev:
                    emit_stage(*stage_prev.pop(0))

            def emit_attention(b, qc, extra=None, last=False):
                # pv trails by two pairs (one on the final qc so only a
                # single pair separates the last exp from the collective)
                lag = 2 if last else 5
                for jj in range(NJP):
                    emit_scores(b, qc, 2 * jj)
                    emit_exp(b, qc, 2 * jj)
                    emit_scores(b, qc, 2 * jj + 1)
                    emit_exp(b, qc, 2 * jj + 1)
                    if jj >= lag:
                        emit_pv(b, qc, jj - lag)
                    if extra is not None:
                        for fn in extra.get(jj, ()):
                            fn()
                for jj in range(NJP - lag, NJP):
                    emit_pv(b, qc, jj)
                if last:
                    while stage_prev:
                        emit_stage_prev()
                    emit_stage(b, qc)
                else:
                    stage_prev.append((b, qc))

            # qc1 hosts chunk 3's V work, qc0's eight PV pairs and its
            # stage; later qc slots drain the batch-1 filler queue
            extra1 = {
                0: [lambda: emit_proj_v_half(0, NRWB - 1, 0),
                    lambda: emit_pv(0, 0, 0)],
                1: [lambda: emit_proj_v_half(0, NRWB - 1, 1),
                    lambda: emit_pv(0, 0, 1)],
                2: [lambda: emit_vtrans(0, NRWB - 1),
                    lambda: emit_pv(0, 0, 2), lambda: emit_pv(0, 0, 3)],
                3: [lambda: emit_pv(0, 0, 4), lambda: emit_pv(0, 0, 5),
                    lambda: emit_pv(0, 0, 6)],
                4: [lambda: emit_pv(0, 0, 7), lambda: emit_stage(0, 0)],
                6: [emit_filler_slice],
                7: [emit_filler_slice],
            }
            emit_attention(0, 1, extra=extra1)

            for qc in range(2, NQC):
                ex = {0: [emit_stage_prev]}
                for jj in range(1, NJP):
                    ex[jj] = [emit_filler_slice]
                emit_attention(0, qc, extra=ex)
            while stage_prev:
                emit_stage_prev()
            emit_collective(0)
            emit_chunk_load(0)
            emit_wo_loads()

            for b in range(1, B):
                while fill_q:
                    emit_filler_slice()
                for qc in range(NQC):
                    ex = {0: [emit_stage_prev]} if stage_prev else None
                    emit_attention(b, qc, extra=ex,
                                   last=(qc == NQC - 1))
                if b < B - 1:
                    emit_collective(b)
                    emit_chunk_load(b)
            psd.__exit__(None, None, None)

            # ---- output projection ----
            # psum pool swaps after the last stage; batch B-1's collective
            # is emitted after the swap so batch 0's projection overlaps it
            # (the pool-close barrier would otherwise order it behind the
            # collective)
            psf = tc.tile_pool(name="ps_f", bufs=1, space="PSUM")
            ps = psf.__enter__()
            ops = {do: ps.tile([P, B * SC], F32, tag="oproj", bufs=8,
                               name=f"ops_{do}") for do in range(NDO)}

            def emit_oproj(b):
                # do-outer so each psum bank finishes early and its bias
                # add + store pipeline behind the remaining matmuls
                otb = work.tile([P, NDO, SC], F32, tag=f"otall{b}", bufs=1,
                                name=f"ot_all_{b}")
                for do in range(NDO):
                    for i in range(NDO):
                        nc.tensor.matmul(
                            ops[do][:, b * SC:(b + 1) * SC],
                            wo_all[:, i, do * P:(do + 1) * P],
                            chunk_sb[b][:, i, :],
                            start=(i == 0), stop=(i == NDO - 1))
                    nc.vector.tensor_scalar_add(
                        otb[:, do, :], ops[do][:, b * SC:(b + 1) * SC],
                        bo_sb[:, do:do + 1])
                dst = outT.ap()[:, b * SC:(b + 1) * SC].rearrange(
                    "(dd p) n -> p dd n", p=P)
                for hh in range(2):
                    dd = slice(hh * NDO // 2, (hh + 1) * NDO // 2)
                    nc.sync.dma_start(dst[:, dd, :], otb[:, dd, :])

            for b in range(B - 1):
                emit_oproj(b)
            emit_collective(B - 1)
            emit_chunk_load(B - 1)
            emit_oproj(B - 1)
            psf.__exit__(None, None, None)

    nc.finalize()
    return nc


# ---------------- host side ----------------

_NC_CACHE = {}

B, S, D, H = 2, 2048, 1024, 16
NCORES = 8


def _q8(a, dtype):
    return np.ascontiguousarray(a).astype(dtype)


def _prep_inputs(x, Wq, bq, Wk, bk, Wv, bv, Wo, bo, ncores):
    Dl = x.shape[-1]
    R = x.shape[0] * x.shape[1]
    FPC = Dl // ncores
    NDO = Dl // P
    xT = np.ascontiguousarray(x.reshape(R, Dl).T)
    x8T = _q8(xT, E4)
    rx8T = _q8(xT - x8T.astype(np.float32), E5)
    woT = _q8((Wo / WSCALE).T, BF)
    bo_eff = bo + Wo @ bv
    bo2 = np.ascontiguousarray(bo_eff.reshape(NDO, P).T.astype(np.float32))
    identm = np.eye(P, dtype=BF)
    maps = []
    for c in range(ncores):
        fsl = slice(c * FPC, (c + 1) * FPC)
        wqkvT = np.ascontiguousarray(
            (WSCALE * np.concatenate([Wq[fsl], Wk[fsl], Wv[fsl]],
                                     axis=0)).T)
        w8 = _q8(wqkvT, E4)
        rw8 = _q8(wqkvT - w8.astype(np.float32), E5)
        bq16 = np.ascontiguousarray(
            (WSCALE * bq[fsl]).reshape(FPC, 1).astype(np.float32))
        maps.append(dict(x8T=x8T, rx8T=rx8T, w8=w8, rw8=rw8, bq16=bq16,
                         woT=woT, bo2=bo2, identb=identm))
    return maps


def kernel(x, Wq, bq, Wk, bk, Wv, bv, Wo, bo):
    from concourse.bass_utils import run_bass_kernel_spmd

    args = [np.asarray(a, np.float32)
            for a in (x, Wq, bq, Wk, bk, Wv, bv, Wo, bo)]
    x = args[0]
    Bx, Sx, Dx = x.shape
    key = (Bx, Sx, Dx)
    if key not in _NC_CACHE:
        _NC_CACHE[key] = build_nc(B=Bx, S=Sx, D=Dx, H=H, ncores=NCORES)
    nc = _NC_CACHE[key]

    in_maps = _prep_inputs(*args, NCORES)
    trace = os.environ.get("KERNEL_TRACE", "0") == "1"
    try:
        res = run_bass_kernel_spmd(nc, in_maps, core_ids=list(range(NCORES)),
                                   trace=trace)
    except ModuleNotFoundError:
        res = run_bass_kernel_spmd(nc, in_maps, core_ids=list(range(NCORES)),
                                   trace=False)
    kernel._last_results = res
    Sc = Sx // NCORES
    out = np.empty((Bx * Sx, Dx), np.float32)
    for c in range(NCORES):
        oc = res.results[c]["outT"].T  # [B*Sc, D]
        for b2 in range(Bx):
            out[b2 * Sx + c * Sc:b2 * Sx + (c + 1) * Sc] = \
                oc[b2 * Sc:(b2 + 1) * Sc]
    return np.ascontiguousarray(out).reshape(Bx, Sx, Dx)
